# revision 10
# baseline (speedup 1.0000x reference)
"""Trainium2 Bass kernel for nn_MultiHeadHighLevelAllocator.

Math (reference):
    uav_embed = MLP_u(uav_feat)                     # (U=256, E=128)
    task_embed = MLP_t(task_feat)                   # (T=512, E=128)
    uq[h,u,:]  = uav_embed[u] + head_queries[h]     # (H=4, U, E)
    a[hu,k]    = uq[hu] @ Wu.T + fb0                # Wu = fw0[:, :E]
    b[t,k]     = task_embed[t] @ Wt.T               # Wt = fw0[:, E:]
    logits[hu,t] = sum_k fw1[k] * relu(a[hu,k] + b[t,k]) + fb1

Strategy (8 cores, shard T -> 64 t's per core, full HU on every core):
    - Prep matmuls on PE in feature-on-partition layout (host pre-transposes
      inputs); a = uqT@Wu + fb0 is evicted straight to fp16 as a16[k, hu]
      (2 k-tiles of (128, 1024)); b[k, t_local] stays fp32 ((128, 64)/k-tile).
    - Fused bias+ReLU per (t, ktile) unit over the (128k, 1024hu) plane:
      ACT relu-with-bias (HW ~1.2us/unit) or DVE tensor_scalar add+max
      (fp16 2x mode, HW ~0.53us/unit); 28/100 static split (empirically
      tuned via interleaved on-HW sweeps).
    - Contraction with fw1 on PE in fp16: lhsT = fw1 k-slice (128,1),
      rhs = R (128,512) x2 halves, M=1 outputs col-tiled to PSUM partitions
      {0,32,64,96} (4 t's per round run concurrently in separate column
      groups: HW-measured 86ns/matmul vs 216ns serial), accumulated over
      the 2 k-tiles. DVE-produced R's are consumed first (j order 1,2,3,0)
      so PE never head-of-line blocks on the slower ACT units.
    - 2-round (128, 2048) PSUM groups; ACT evicts (+fb1) one group late to
      avoid blocking unit production; strided-row DMA gathers the 4 valid
      partitions per sub-round.

Output per core: (64, 1024) fp32 [t_local, h*U+u]; host reassembles (H,U,T).
"""

import contextlib

import numpy as np

import concourse.bacc as bacc
import concourse.mybir as mybir
from concourse.tile import TileContext
from concourse.bass_utils import run_bass_kernel_spmd

U, T, H = 256, 512, 4
UAV_DIM, TASK_DIM, E, HID = 64, 32, 128, 256
HU = H * U                      # 1024
NCORES = 8
TL = T // NCORES                # 64 t's per core
NKT = HID // 128                # 2 k-tiles
NROUNDS = TL // 4               # 16 rounds of 4 t's

f32 = mybir.dt.float32
f16 = mybir.dt.float16
f32r = mybir.dt.float32r
AF = mybir.ActivationFunctionType
ALU = mybir.AluOpType
ET = mybir.EngineType

# Tunables; _get_nc caches on their values.
#   x: total ACT R-units (of 128); z: total Pool/GPSIMD R-units; rest DVE.
#   rpool: R-tile pool depth; evict_dve: # of the 8 evictions done on DVE
CFG = {"x": 19, "z": 28, "rpool": 48, "prep_act": 0, "evict_dve": 0}

_UNIT_ORDER = [(0, 0), (0, 1), (1, 0), (1, 1), (2, 0), (2, 1), (3, 0), (3, 1)]


def _units_for_round(r):
    """Map each of the 8 (j, kt) units of round r to an engine tag.

    ACT gets the first slots, Pool the last, DVE the middle; totals x/z
    are spread evenly across the 16 rounds via cumulative counts."""
    x, z = CFG["x"], CFG["z"]
    na = ((r + 1) * x) // NROUNDS - (r * x) // NROUNDS
    np_ = ((r + 1) * z) // NROUNDS - (r * z) // NROUNDS
    na = min(na, 8)
    np_ = min(np_, 8 - na)
    eng = {}
    for i, u in enumerate(_UNIT_ORDER):
        if i < na:
            eng[u] = "act"
        elif i >= 8 - np_:
            eng[u] = "pool"
        else:
            eng[u] = "dve"
    return eng

IN_SPECS = [
    ("uavT", (UAV_DIM, U), f32),
    ("uw0T", (UAV_DIM, 128), f32),
    ("uw1T", (128, 128), f32),
    ("uw2T", (128, E), f32),
    ("ub0c", (128, 1), f32),
    ("ub1c", (128, 1), f32),
    ("hq2T", (E, H), f32),
    ("taskT", (TASK_DIM, TL), f32),
    ("tw0T", (TASK_DIM, 128), f32),
    ("tw1T", (128, 128), f32),
    ("tw2T", (128, E), f32),
    ("tb0c", (128, 1), f32),
    ("tb1c", (128, 1), f32),
    ("tb2c", (128, 1), f32),
    ("WuT", (E, HID), f32),
    ("WtT", (E, HID), f32),
    ("fb0c", (128, NKT), f32),
    ("fw1c", (128, NKT), f16),
]


def _emit_loads(nc, d, singles):
    s = {}
    for name, shape, dt_ in IN_SPECS:
        s[name] = singles.tile(list(shape), dt_, name=name, tag=name)
        nc.sync.dma_start(out=s[name], in_=d[name][:])
    return s


def _emit_body(nc, d, s, pools, mult):
    singles, prep, ppsum, rpool, opool, fpsum = pools

    # ---- encoders + a/b prep ----
    uqT_s = singles.tile([E, HU], f32, name="uqT", tag="uqT")
    a16_s = [singles.tile([128, HU], f16, tag=f"a16_{kt}", name=f"a16_{kt}")
             for kt in range(NKT)]
    b_s = [singles.tile([128, TL], f32, tag=f"b{kt}", name=f"b{kt}")
           for kt in range(NKT)]

    # uav + task encoders, chains interleaved so PE/ACT ping-pong.
    pe1 = ppsum.tile([128, U], f32, tag="ps_o", name="pe1")
    nc.tensor.matmul(pe1, s["uw0T"], s["uavT"], start=True, stop=True)
    pt1 = ppsum.tile([128, TL], f32, tag="ps_o", name="pt1")
    nc.tensor.matmul(pt1, s["tw0T"], s["taskT"], start=True, stop=True)
    h1 = prep.tile([128, U], f32, tag="pr", name="h1")
    nc.scalar.activation(h1, pe1, AF.Relu, bias=s["ub0c"][:, 0:1])
    s1 = prep.tile([128, TL], f32, tag="pr", name="s1")
    nc.scalar.activation(s1, pt1, AF.Relu, bias=s["tb0c"][:, 0:1])
    pe2 = ppsum.tile([128, U], f32, tag="ps_o", name="pe2")
    nc.tensor.matmul(pe2, s["uw1T"], h1, start=True, stop=True)
    pt2 = ppsum.tile([128, TL], f32, tag="ps_o", name="pt2")
    nc.tensor.matmul(pt2, s["tw1T"], s1, start=True, stop=True)
    h2 = prep.tile([128, U], f32, tag="pr", name="h2")
    nc.scalar.activation(h2, pe2, AF.Relu, bias=s["ub1c"][:, 0:1])
    s2 = prep.tile([128, TL], f32, tag="pr", name="s2")
    nc.scalar.activation(s2, pt2, AF.Relu, bias=s["tb1c"][:, 0:1])
    pe3 = ppsum.tile([E, U], f32, tag="ps_o", name="pe3")
    nc.tensor.matmul(pe3, s["uw2T"], h2, start=True, stop=True)
    pt3 = ppsum.tile([E, TL], f32, tag="ps_o", name="pt3")
    nc.tensor.matmul(pt3, s["tw2T"], s2, start=True, stop=True)
    # uqT[:, h-block] = uav_embedT + (head_queries[h] + ub2)   (ACT; f32r out)
    for h in range(H):
        nc.scalar.activation(
            uqT_s[:, h * U : (h + 1) * U], pe3, AF.Identity,
            bias=s["hq2T"][:, h : h + 1],
        )
    teT = prep.tile([E, TL], f32, tag="pr", name="teT")
    nc.scalar.activation(teT, pt3, AF.Identity, bias=s["tb2c"][:, 0:1])

    # b[kt] = (WtT slice).T @ teT  -> (128, TL)
    for kt in range(NKT):
        pb = ppsum.tile([128, TL], f32, tag="ps_o", name=f"pb{kt}")
        nc.tensor.matmul(pb, s["WtT"][:, kt * 128 : (kt + 1) * 128], teT,
                         start=True, stop=True)
        if CFG["prep_act"]:
            nc.scalar.copy(out=b_s[kt], in_=pb)
        else:
            nc.vector.tensor_copy(out=b_s[kt], in_=pb)

    # a[kt] = (WuT slice).T @ uqT + fb0  -> (128, HU)
    for kt in range(NKT):
        for half in range(2):
            pa = ppsum.tile([128, 512], f32, tag="ps_o", name=f"pa{kt}{half}")
            nc.tensor.matmul(
                pa, s["WuT"][:, kt * 128 : (kt + 1) * 128],
                uqT_s[:, half * 512 : (half + 1) * 512],
                start=True, stop=True,
            )
            nc.scalar.activation(
                a16_s[kt][:, half * 512 : (half + 1) * 512], pa,
                AF.Identity, bias=s["fb0c"][:, kt : kt + 1],
            )

    # ---- fusion: 8 groups of 2 rounds; evictions delayed one group ----
    # (fb1 is added on the host during the gather.)
    NG = NROUNDS // 2
    pending = []        # (group_idx, psum_tile)

    def evict(gg, ps):
        g = gg % NG
        o_st = opool.tile([128, 2 * HU], f32, tag="o", name=f"o{gg}")
        if (gg % NG) % 8 < CFG["evict_dve"]:
            nc.vector.tensor_copy(out=o_st, in_=ps)
        else:
            nc.scalar.copy(out=o_st, in_=ps)
        osrc = o_st.rearrange("(j i) (sub n) -> sub j i n", j=4, sub=2)
        for sub in range(2):
            nc.sync.dma_start(
                out=d["out"][8 * g + 4 * sub : 8 * g + 4 * sub + 4, :],
                in_=osrc[sub, :, 0, :],
            )

    _RANK = {"dve": 0, "act": 1, "pool": 2}
    for gg in range(NG * mult):
        g = gg % NG
        ps_g = fpsum.tile([128, 2 * HU], f32, tag="ps_o", name=f"ps_g{gg}")
        for sub in range(2):
            r = 2 * g + sub
            eng = _units_for_round(r)
            rt = {}
            for kt in range(NKT):
                for j in range(4):
                    t = 4 * r + j
                    Rt = rpool.tile([128, HU], f16, tag="R",
                                    name=f"R{gg}_{sub}_{j}_{kt}")
                    bias_ap = b_s[kt][:, t : t + 1]
                    e = eng[(j, kt)]
                    if e == "act":
                        nc.scalar.activation(Rt, a16_s[kt], AF.Relu,
                                             bias=bias_ap)
                    elif e == "pool":
                        nc.gpsimd.tensor_scalar(
                            out=Rt, in0=a16_s[kt], scalar1=bias_ap,
                            scalar2=0.0, op0=ALU.add, op1=ALU.max,
                        )
                    else:
                        nc.vector.tensor_scalar(
                            out=Rt, in0=a16_s[kt], scalar1=bias_ap,
                            scalar2=0.0, op0=ALU.add, op1=ALU.max,
                        )
                    rt[(j, kt)] = Rt
            # contraction: consume fast producers first so PE never
            # head-of-line blocks on ACT/Pool units
            for kt in range(NKT):
                order = sorted(range(4), key=lambda j: _RANK[eng[(j, kt)]])
                for half in range(2):
                    for j in order:
                        nc.tensor.matmul(
                            ps_g[32 * j : 32 * j + 1,
                                 sub * HU + half * 512 :
                                 sub * HU + (half + 1) * 512],
                            s["fw1c"][:, kt : kt + 1],
                            rt[(j, kt)][:, half * 512 : (half + 1) * 512],
                            start=(kt == 0), stop=(kt == NKT - 1),
                            tile_position=(0, 32 * j),
                        )
        pending.append((gg, ps_g))
        if len(pending) > 1:
            evict(*pending.pop(0))
    while pending:
        evict(*pending.pop(0))


def _build_nc(mult=1, loop=None):
    nc = bacc.Bacc(None, target_bir_lowering=False)
    d = {}
    for name, shape, dt_ in IN_SPECS:
        d[name] = nc.dram_tensor(name, list(shape), dt_, kind="ExternalInput")
    d["out"] = nc.dram_tensor("out", [TL, HU], f32, kind="ExternalOutput")

    with TileContext(nc) as tc:
        with tc.tile_pool(name="singles", bufs=1) as singles, \
             tc.tile_pool(name="prep", bufs=2) as prep, \
             tc.tile_pool(name="rpool", bufs=CFG["rpool"]) as rpool, \
             tc.tile_pool(name="opool", bufs=3) as opool, \
             tc.tile_pool(name="fpsum", bufs=2, space="PSUM") as fpsum:
            pools = (singles, prep, fpsum, rpool, opool, fpsum)
            s = _emit_loads(nc, d, singles)
            ctx = (tc.For_i(0, loop, 1,
                            hint_engines=(ET.PE, ET.Activation, ET.DVE,
                                          ET.Pool))
                   if loop else contextlib.nullcontext())
            with ctx:
                _emit_body(nc, d, s, pools, mult)

    nc.finalize()
    return nc


_NC_CACHE = {}


def _get_nc(mult=1, loop=None):
    key = (mult, loop, tuple(sorted(CFG.items())))
    if key not in _NC_CACHE:
        _NC_CACHE[key] = _build_nc(mult, loop)
    return _NC_CACHE[key]


def _prep_inputs(inputs):
    ct = np.ascontiguousarray
    f = np.float32
    uav_feat = inputs["uav_feat"].astype(f)
    task_feat = inputs["task_feat"].astype(f)
    base = {
        "uavT": ct(uav_feat.T),
        "uw0T": ct(inputs["uw0"].T.astype(f)),
        "uw1T": ct(inputs["uw1"].T.astype(f)),
        "uw2T": ct(inputs["uw2"].T.astype(f)),
        "ub0c": ct(inputs["ub0"].astype(f).reshape(128, 1)),
        "ub1c": ct(inputs["ub1"].astype(f).reshape(128, 1)),
        "hq2T": ct((inputs["head_queries"].astype(f)
                    + inputs["ub2"].astype(f)[None, :]).T),
        "tw0T": ct(inputs["tw0"].T.astype(f)),
        "tw1T": ct(inputs["tw1"].T.astype(f)),
        "tw2T": ct(inputs["tw2"].T.astype(f)),
        "tb0c": ct(inputs["tb0"].astype(f).reshape(128, 1)),
        "tb1c": ct(inputs["tb1"].astype(f).reshape(128, 1)),
        "tb2c": ct(inputs["tb2"].astype(f).reshape(128, 1)),
        "WuT": ct(inputs["fw0"][:, :E].T.astype(f)),
        "WtT": ct(inputs["fw0"][:, E:].T.astype(f)),
        "fb0c": ct(inputs["fb0"].astype(f).reshape(NKT, 128).T),
        "fw1c": ct(inputs["fw1"].reshape(NKT, 128).T.astype(np.float16)),
    }
    taskT_full = ct(task_feat.T)
    in_maps = []
    for c in range(NCORES):
        m = dict(base)
        m["taskT"] = ct(taskT_full[:, c * TL : (c + 1) * TL])
        in_maps.append(m)
    return in_maps


def run(trace=False, **inputs):
    nc = _get_nc()
    in_maps = _prep_inputs(inputs)
    res = run_bass_kernel_spmd(nc, in_maps, list(range(NCORES)), trace=trace)
    big = np.concatenate([res.results[c]["out"] for c in range(NCORES)], axis=0)
    out = np.ascontiguousarray(big.T).reshape(H, U, T) + np.float32(
        inputs["fb1"][0]
    )
    return out, res


def kernel(**inputs):
    out, _ = run(**inputs)
    return out



# revision 11
# speedup vs baseline: 1.0408x; 1.0408x over previous
"""Trainium2 Bass kernel for nn_MultiHeadHighLevelAllocator.

Math (reference):
    uav_embed = MLP_u(uav_feat)                     # (U=256, E=128)
    task_embed = MLP_t(task_feat)                   # (T=512, E=128)
    uq[h,u,:]  = uav_embed[u] + head_queries[h]     # (H=4, U, E)
    a[hu,k]    = uq[hu] @ Wu.T + fb0                # Wu = fw0[:, :E]
    b[t,k]     = task_embed[t] @ Wt.T               # Wt = fw0[:, E:]
    logits[hu,t] = sum_k fw1[k] * relu(a[hu,k] + b[t,k]) + fb1

Strategy (8 cores, shard T -> 64 t's per core, full HU on every core):
    - All affine-foldable terms are precomputed on the host into one
      per-(kt,h) bias column dhq[k, kt*H+h] = Wu@(hq_h+ub2) + fb0 + Wt@tb2,
      so the device only computes ue_raw = MLP_u's linear part and
      a16[k, h*U+u] = (Wu@ue_raw)[k,u] + dhq (ACT evict, fp16), plus
      b[k,t] = (Wt@te_raw)[k,t] (DVE evict, fp32).
    - Inputs are packed into two 128-partition f32 blocks: par1 (encoder
      path) on the SP DMA queue, par2 (fusion weights) on the ACT queue
      in parallel -> last weight lands ~2us instead of ~7us of serial
      small DMAs. ACT's function table is warmed at t=0.
    - Fused bias+ReLU per (t, ktile) unit over the (128k, 1024hu) plane:
      ACT relu-with-bias (HW ~1.2us/unit) or DVE tensor_scalar add+max
      (fp16 2x mode, HW ~0.53us/unit); x/128 static split, HW-tuned.
    - Contraction with fw1 on PE in fp16: lhsT = fw1 k-slice (128,1),
      rhs = R (128,512) x2 halves, M=1 outputs col-tiled to PSUM partitions
      {0,32,64,96} (4 t's per round run concurrently in separate column
      groups: HW-measured 86ns/matmul vs 216ns serial), accumulated over
      the 2 k-tiles. DVE-produced R's are consumed first so PE never
      head-of-line blocks on the slower ACT units.
    - 2-round (128, 2048) PSUM groups; eviction (plain copy; fb1 is added
      on the host) one group late; the last group's eviction is split
      ACT||DVE to shorten the drain. Strided-row DMA gathers the 4 valid
      partitions per sub-round.

Output per core: (64, 1024) fp32 [t_local, h*U+u]; host adds fb1 and
reassembles (H,U,T).
"""

import contextlib

import numpy as np

import concourse.bacc as bacc
import concourse.mybir as mybir
from concourse.tile import TileContext
from concourse.bass_utils import run_bass_kernel_spmd

U, T, H = 256, 512, 4
UAV_DIM, TASK_DIM, E, HID = 64, 32, 128, 256
HU = H * U                      # 1024
NCORES = 8
TL = T // NCORES                # 64 t's per core
NKT = HID // 128                # 2 k-tiles
NROUNDS = TL // 4               # 16 rounds of 4 t's

f32 = mybir.dt.float32
f16 = mybir.dt.float16
AF = mybir.ActivationFunctionType
ALU = mybir.AluOpType
ET = mybir.EngineType

# Tunables; _get_nc caches on their values.
#   x: total ACT R-units (of 128); rest DVE. rpool: R-tile pool depth.
#   evict_dve: # of the 8 evictions done on DVE
CFG = {"x": 28, "rpool": 48, "evict_dve": 0, "split_last": 1}

_UNIT_ORDER = [(0, 0), (0, 1), (1, 0), (1, 1), (2, 0), (2, 1), (3, 0), (3, 1)]


def _units_for_round(r):
    """Map each of the 8 (j, kt) units of round r to an engine tag.

    ACT gets the first slots, DVE the rest; the total x is spread evenly
    across the 16 rounds via cumulative counts."""
    x = CFG["x"]
    na = min(((r + 1) * x) // NROUNDS - (r * x) // NROUNDS, 8)
    eng = {}
    for i, u in enumerate(_UNIT_ORDER):
        eng[u] = "act" if i < na else "dve"
    return eng


# --- packed input layout (par1: encoder path, par2: fusion weights) ----
P1 = {}
_c = 0
for _name, _w in [("uavT", U), ("uw0T", 128), ("ub0", 1), ("uw1T", 128),
                  ("ub1", 1), ("uw2T", E), ("taskT", TL), ("tw0T", 128),
                  ("tb0", 1), ("tw1T", 128), ("tb1", 1), ("tw2T", E)]:
    P1[_name] = (_c, _c + _w)
    _c += _w
P1W = _c                        # 1092
P2 = {}
_c = 0
for _name, _w in [("WuT", HID), ("WtT", HID), ("dhq", NKT * H)]:
    P2[_name] = (_c, _c + _w)
    _c += _w
P2W = _c                        # 520

IN_SPECS = [
    ("par1", (128, P1W), f32),
    ("par2", (128, P2W), f32),
    ("fw1c", (128, NKT), f16),
]


def _emit_loads(nc, d, singles):
    s = {}
    for name, shape, dt_ in IN_SPECS:
        s[name] = singles.tile(list(shape), dt_, name=name, tag=name)
    # par2 + fw1c on the ACT HWDGE queue, par1 on SP: both issue at t=0
    nc.sync.dma_start(out=s["par1"], in_=d["par1"][:])
    nc.scalar.dma_start(out=s["par2"], in_=d["par2"][:])
    nc.scalar.dma_start(out=s["fw1c"], in_=d["fw1c"][:])
    return s


def _emit_body(nc, d, s, pools, mult):
    singles, prep, ppsum, rpool, opool, fpsum = pools
    p1, p2 = s["par1"], s["par2"]

    def c1(name, rows=128):
        lo, hi = P1[name]
        return p1[:rows, lo:hi]

    def c2(name):
        lo, hi = P2[name]
        return p2[:, lo:hi]

    # warm the ACT function table while input DMAs are in flight
    warm = singles.tile([128, 1], f32, name="warm", tag="warm")
    nc.vector.memset(warm, 0.0)
    warm2 = singles.tile([128, 1], f32, name="warm2", tag="warm2")
    nc.scalar.activation(warm2, warm, AF.Relu, bias=warm[:, 0:1])

    a16_s = [singles.tile([128, HU], f16, tag=f"a16_{kt}", name=f"a16_{kt}")
             for kt in range(NKT)]
    b_s = [singles.tile([128, TL], f32, tag=f"b{kt}", name=f"b{kt}")
           for kt in range(NKT)]

    # uav + task encoders, chains interleaved so PE/ACT ping-pong.
    pe1 = ppsum.tile([128, U], f32, tag="ps_o", name="pe1")
    nc.tensor.matmul(pe1, c1("uw0T", UAV_DIM), c1("uavT", UAV_DIM),
                     start=True, stop=True)
    pt1 = ppsum.tile([128, TL], f32, tag="ps_o", name="pt1")
    nc.tensor.matmul(pt1, c1("tw0T", TASK_DIM), c1("taskT", TASK_DIM),
                     start=True, stop=True)
    h1 = prep.tile([128, U], f32, tag="pr", name="h1")
    nc.scalar.activation(h1, pe1, AF.Relu, bias=c1("ub0"))
    s1 = prep.tile([128, TL], f32, tag="pr", name="s1")
    nc.scalar.activation(s1, pt1, AF.Relu, bias=c1("tb0"))
    pe2 = ppsum.tile([128, U], f32, tag="ps_o", name="pe2")
    nc.tensor.matmul(pe2, c1("uw1T"), h1, start=True, stop=True)
    pt2 = ppsum.tile([128, TL], f32, tag="ps_o", name="pt2")
    nc.tensor.matmul(pt2, c1("tw1T"), s1, start=True, stop=True)
    h2 = prep.tile([128, U], f32, tag="pr", name="h2")
    nc.scalar.activation(h2, pe2, AF.Relu, bias=c1("ub1"))
    s2 = prep.tile([128, TL], f32, tag="pr", name="s2")
    nc.scalar.activation(s2, pt2, AF.Relu, bias=c1("tb1"))
    pe3 = ppsum.tile([E, U], f32, tag="ps_o", name="pe3")
    nc.tensor.matmul(pe3, c1("uw2T"), h2, start=True, stop=True)
    pt3 = ppsum.tile([E, TL], f32, tag="ps_o", name="pt3")
    nc.tensor.matmul(pt3, c1("tw2T"), s2, start=True, stop=True)
    # linear encoder outputs (biases folded into dhq on the host)
    ueT = prep.tile([E, U], f32, tag="pr", name="ueT")
    nc.vector.tensor_copy(out=ueT, in_=pe3)
    teT = prep.tile([E, TL], f32, tag="pr", name="teT")
    nc.vector.tensor_copy(out=teT, in_=pt3)

    # a16[kt][:, h*U:(h+1)*U] = (WuT slice).T @ ueT + dhq[:, kt*H+h]
    for kt in range(NKT):
        pa = ppsum.tile([128, U], f32, tag="ps_o", name=f"pa{kt}")
        nc.tensor.matmul(pa, c2("WuT")[:, kt * 128:(kt + 1) * 128], ueT,
                         start=True, stop=True)
        dlo = P2["dhq"][0] + kt * H
        for h in range(H):
            nc.scalar.activation(
                a16_s[kt][:, h * U:(h + 1) * U], pa, AF.Identity,
                bias=p2[:, dlo + h:dlo + h + 1],
            )
        # b[kt] = (WtT slice).T @ teT  -> (128, TL)
        pb = ppsum.tile([128, TL], f32, tag="ps_o", name=f"pb{kt}")
        nc.tensor.matmul(pb, c2("WtT")[:, kt * 128:(kt + 1) * 128], teT,
                         start=True, stop=True)
        nc.vector.tensor_copy(out=b_s[kt], in_=pb)

    # ---- fusion: 8 groups of 2 rounds; evictions delayed one group ----
    # (fb1 is added on the host during the gather.)
    NG = NROUNDS // 2
    pending = []        # (group_idx, psum_tile)

    def evict(gg, ps, split=False):
        g = gg % NG
        o_st = opool.tile([128, 2 * HU], f32, tag="o", name=f"o{gg}")
        if split:
            nc.scalar.copy(out=o_st[:, 0:HU], in_=ps[:, 0:HU])
            nc.vector.tensor_copy(out=o_st[:, HU:], in_=ps[:, HU:])
        elif (gg % NG) % 8 < CFG["evict_dve"]:
            nc.vector.tensor_copy(out=o_st, in_=ps)
        else:
            nc.scalar.copy(out=o_st, in_=ps)
        osrc = o_st.rearrange("(j i) (sub n) -> sub j i n", j=4, sub=2)
        for sub in range(2):
            nc.sync.dma_start(
                out=d["out"][8 * g + 4 * sub : 8 * g + 4 * sub + 4, :],
                in_=osrc[sub, :, 0, :],
            )

    for gg in range(NG * mult):
        g = gg % NG
        ps_g = fpsum.tile([128, 2 * HU], f32, tag="ps_o", name=f"ps_g{gg}")
        for sub in range(2):
            r = 2 * g + sub
            eng = _units_for_round(r)
            rt = {}
            for kt in range(NKT):
                for j in range(4):
                    t = 4 * r + j
                    Rt = rpool.tile([128, HU], f16, tag="R",
                                    name=f"R{gg}_{sub}_{j}_{kt}")
                    bias_ap = b_s[kt][:, t : t + 1]
                    if eng[(j, kt)] == "act":
                        nc.scalar.activation(Rt, a16_s[kt], AF.Relu,
                                             bias=bias_ap)
                    else:
                        nc.vector.tensor_scalar(
                            out=Rt, in0=a16_s[kt], scalar1=bias_ap,
                            scalar2=0.0, op0=ALU.add, op1=ALU.max,
                        )
                    rt[(j, kt)] = Rt
            # contraction: consume DVE-produced R's first so PE never
            # head-of-line blocks on the slower ACT units
            for kt in range(NKT):
                order = sorted(range(4),
                               key=lambda j: 0 if eng[(j, kt)] == "dve" else 1)
                for half in range(2):
                    for j in order:
                        nc.tensor.matmul(
                            ps_g[32 * j : 32 * j + 1,
                                 sub * HU + half * 512 :
                                 sub * HU + (half + 1) * 512],
                            s["fw1c"][:, kt : kt + 1],
                            rt[(j, kt)][:, half * 512 : (half + 1) * 512],
                            start=(kt == 0), stop=(kt == NKT - 1),
                            tile_position=(0, 32 * j),
                        )
        pending.append((gg, ps_g))
        if len(pending) > 1:
            evict(*pending.pop(0))
    while pending:
        gg, ps = pending.pop(0)
        evict(gg, ps, split=bool(CFG["split_last"]) and not pending)


def _build_nc(mult=1, loop=None):
    nc = bacc.Bacc(None, target_bir_lowering=False)
    d = {}
    for name, shape, dt_ in IN_SPECS:
        d[name] = nc.dram_tensor(name, list(shape), dt_, kind="ExternalInput")
    d["out"] = nc.dram_tensor("out", [TL, HU], f32, kind="ExternalOutput")

    with TileContext(nc) as tc:
        with tc.tile_pool(name="singles", bufs=1) as singles, \
             tc.tile_pool(name="prep", bufs=2) as prep, \
             tc.tile_pool(name="rpool", bufs=CFG["rpool"]) as rpool, \
             tc.tile_pool(name="opool", bufs=3) as opool, \
             tc.tile_pool(name="fpsum", bufs=2, space="PSUM") as fpsum:
            pools = (singles, prep, fpsum, rpool, opool, fpsum)
            s = _emit_loads(nc, d, singles)
            ctx = (tc.For_i(0, loop, 1,
                            hint_engines=(ET.PE, ET.Activation, ET.DVE))
                   if loop else contextlib.nullcontext())
            with ctx:
                _emit_body(nc, d, s, pools, mult)

    nc.finalize()
    return nc


_NC_CACHE = {}


def _get_nc(mult=1, loop=None):
    key = (mult, loop, tuple(sorted(CFG.items())))
    if key not in _NC_CACHE:
        _NC_CACHE[key] = _build_nc(mult, loop)
    return _NC_CACHE[key]


def _prep_inputs(inputs):
    ct = np.ascontiguousarray
    f = np.float32
    uav_feat = inputs["uav_feat"].astype(f)
    task_feat = inputs["task_feat"].astype(f)
    Wu = inputs["fw0"][:, :E].astype(f)          # (HID, E)
    Wt = inputs["fw0"][:, E:].astype(f)          # (HID, E)
    hqb = inputs["head_queries"].astype(f) + inputs["ub2"].astype(f)[None, :]
    dh = (Wu @ hqb.T + inputs["fb0"].astype(f)[:, None]
          + (Wt @ inputs["tb2"].astype(f))[:, None])       # (HID, H)
    dhq = np.zeros((128, NKT * H), dtype=f)
    for kt in range(NKT):
        dhq[:, kt * H:(kt + 1) * H] = dh[kt * 128:(kt + 1) * 128, :]

    par1 = np.zeros((128, P1W), dtype=f)

    def put1(name, arr, rows=128):
        lo, hi = P1[name]
        par1[:rows, lo:hi] = arr

    put1("uavT", uav_feat.T, UAV_DIM)
    put1("uw0T", inputs["uw0"].T.astype(f), UAV_DIM)
    put1("ub0", inputs["ub0"].astype(f).reshape(128, 1))
    put1("uw1T", inputs["uw1"].T.astype(f))
    put1("ub1", inputs["ub1"].astype(f).reshape(128, 1))
    put1("uw2T", inputs["uw2"].T.astype(f))
    put1("tw0T", inputs["tw0"].T.astype(f), TASK_DIM)
    put1("tb0", inputs["tb0"].astype(f).reshape(128, 1))
    put1("tw1T", inputs["tw1"].T.astype(f))
    put1("tb1", inputs["tb1"].astype(f).reshape(128, 1))
    put1("tw2T", inputs["tw2"].T.astype(f))

    par2 = np.zeros((128, P2W), dtype=f)
    par2[:, P2["WuT"][0]:P2["WuT"][1]] = Wu.T
    par2[:, P2["WtT"][0]:P2["WtT"][1]] = Wt.T
    par2[:, P2["dhq"][0]:P2["dhq"][1]] = dhq

    fw1c = ct(inputs["fw1"].reshape(NKT, 128).T.astype(np.float16))

    taskT_full = ct(task_feat.T)
    in_maps = []
    for c in range(NCORES):
        p1c = par1.copy()
        lo, hi = P1["taskT"]
        p1c[:TASK_DIM, lo:hi] = taskT_full[:, c * TL:(c + 1) * TL]
        in_maps.append({"par1": ct(p1c), "par2": ct(par2), "fw1c": fw1c})
    return in_maps


def run(trace=False, **inputs):
    nc = _get_nc()
    in_maps = _prep_inputs(inputs)
    res = run_bass_kernel_spmd(nc, in_maps, list(range(NCORES)), trace=trace)
    big = np.concatenate([res.results[c]["out"] for c in range(NCORES)], axis=0)
    out = np.ascontiguousarray(big.T).reshape(H, U, T) + np.float32(
        inputs["fb1"][0]
    )
    return out, res


def kernel(**inputs):
    out, _ = run(**inputs)
    return out
